# revision 10
# baseline (speedup 1.0000x reference)
"""Trainium2 Bass kernel for nn_MemoryCell (scatter_memory), v6.

Full-input contract: kernel(**inputs) takes the complete (unsharded) numpy
inputs and returns the full [NB*B, H] output.

Math (B == H == 1024, NB == 5, T == 128):
    enc  = features[:, 0, :]                         # [B, H] - only slice used
    h    = states.reshape(NB, H)
    gate = sigmoid(enc @ (h + keys).T)               # [B, NB]
    pre  = (h @ Uw.T + keys @ Vw.T)[:, None, :] + (enc @ Ww.T)[None, :, :]
    cand = where(pre >= 0, pre, prelu_a * pre)
    new[i, b, j] = h[i, j] + gate[j, i] * cand[i, b, j]   # B==H broadcast quirk
    out  = sign(new) with exact zeros -> +1, reshaped [NB*B, H]

Because gate > 0 and (for prelu slope a > 0) new is monotone in ew =
enc @ Ww.T, each output element is a pure threshold test:

    sign(new[i, b, j]) = +1  iff  ew[j, b] + nthr[j, i] >= 0
    nthr = huv + (h / s) * (1 + exp(-z)),  s = a if h > 0 else 1

v6 split vs the v5 kernel: the device now ONLY computes ew (the one big
matmul) and ships it back as fp16; the host applies the 5 thresholds.
That removes the ACT-table load, the Sign/is_ge tail ops, and the
threshold input DMAs from the device program, and cuts output bytes
2.5x (2 B/elem instead of 5 sign bytes).  A smaller program also means
fewer tile/walrus semaphores, which directly shortens the fixed
end-of-NEFF semaphore-sweep epilogue (~115 ns per semaphore, measured).

Sharding 2D: 4 j-shards (256 features) x 2 b-halves (512 batch).  Per
core: enc half 1.05 MB + Ww j-shard 0.52 MB in, 0.26 MB ew out.

Device program per core:
  * enc ships in 4 k-split rings (k0 | k1 | k2-3 | k4-7) and Ww in 2
    (k0-1 | k2-7) so the first matmul can start after ~256 KB lands
    while the bulk streams.  SP and Scalar drive the two HWDGE gen
    paths; the k1 ring rides gpsimd's SWDGE as a third path.
  * PE: k-major accumulation into two PSUM banks (one per 128-j group),
    16 matmuls of 512 cols.  A few warm-up transposes on a zeroed tile
    keep the PE clock ramping while the first ring lands.
  * Tail: two DVE tensor_copy casts fp32 PSUM -> fp16 SBUF, then two
    output DMAs (SP + Scalar).
"""

import numpy as np

H = 1024
NB = 5
B = 1024
NCORES = 8
NJ = 4                    # j shards
JS = H // NJ              # 256 features per core (2 PE groups of 128)
HB = B // 2               # 512 batch columns per core
KC = H // 128             # 8 contraction chunks
WARMUP = 12

_NC_CACHE = {}

# enc k-chunk split across rings: ring -> list of k chunks
ENC_RINGS = ((0,), (1,), (2, 3), (4, 5), (6, 7))
WT_RINGS = ((0, 1), (2, 3), (4, 5), (6, 7))


def _make_tc_class():
    import concourse.tile as tile
    from concourse.vector_clock import ScopedClock

    class _LeanTileContext(tile.TileContext):
        """TileContext with a minimal teardown: keep the sync-engine drain
        that waits for every DMA/compute lane to reach its final count (so
        the program cannot retire before the output DMAs complete), but
        skip the two all-engine barriers and the semaphore clears.  Engines
        then retire as soon as their own work ends, so the runtime's fixed
        per-engine semaphore-restore epilogue runs concurrently with the
        kernel tail instead of strictly after it."""

        def _drain_and_barrier(self, tick_clock, wait_clock):
            drain_inst = self.nc.sync.drain()
            wait_clock.add_sem_waits(
                drain_inst.ins, ScopedClock({None: tick_clock.global_clock})
            )
            popped = self.nc._tile_sem_poison_stack.pop()
            assert popped is self._sem_poison

    return _LeanTileContext


def _build_nc():
    from concourse import bacc, mybir
    import concourse.tile as tile

    f32 = mybir.dt.float32
    f16 = mybir.dt.float16

    nc = bacc.Bacc("TRN2", debug=False, num_devices=NCORES)

    enc_d = [nc.dram_tensor(f"e{q}", [128, len(ks), HB], f16,
                            kind="ExternalInput").ap()
             for q, ks in enumerate(ENC_RINGS)]
    wt_d = [nc.dram_tensor(f"w{p}", [128, len(ks), JS], f16,
                           kind="ExternalInput").ap()
            for p, ks in enumerate(WT_RINGS)]
    out_d = [nc.dram_tensor(f"og{g}", [128, HB], f16,
                            kind="ExternalOutput").ap() for g in range(2)]

    with _make_tc_class()(nc) as tc:
        with (
            tc.tile_pool(name="res", bufs=1) as res,
            tc.tile_pool(name="ps", bufs=1, space="PSUM") as ps,
        ):
            warm = res.tile([128, 128], f32, name="warm")
            nc.gpsimd.memset(warm, 0.0)

            enc = [res.tile([128, len(ks), HB], f16, name=f"e{q}")
                   for q, ks in enumerate(ENC_RINGS)]
            wt = [res.tile([128, len(ks), JS], f16, name=f"w{p}")
                  for p, ks in enumerate(WT_RINGS)]
            oe = [res.tile([128, HB], f16, name=f"og{g}") for g in range(2)]

            # the k0 pair (e0 + w01) rides the FIRST trigger slot of each
            # HWDGE engine so the first matmul can start as early as
            # possible; gpsimd's SWDGE (slow ~1us-later completion) gets a
            # mid-schedule ring; the k6/k7 rings land last and feed only
            # the final 4 matmuls.
            nc.scalar.dma_start(enc[0], enc_d[0])
            nc.sync.dma_start(wt[0], wt_d[0])
            nc.scalar.dma_start(enc[1], enc_d[1])
            nc.gpsimd.dma_start(enc[2], enc_d[2])
            nc.scalar.dma_start(wt[1], wt_d[1])
            nc.sync.dma_start(enc[3], enc_d[3])
            nc.scalar.dma_start(wt[2], wt_d[2])
            nc.sync.dma_start(enc[4], enc_d[4])
            nc.scalar.dma_start(wt[3], wt_d[3])

            pw = ps.tile([128, 512], f32, name="pw")
            pg = [ps.tile([128, 512], f32, name=f"pg{g}") for g in range(2)]

            # PE warm-up on the zeroed tile: keeps the clock ramping while
            # the first enc/wt rings stream
            for _ in range(WARMUP):
                nc.tensor.transpose(pw[:, 0:128], warm, warm)

            # ew[j, b] = sum_k Ww[j, k] enc[b, k], k-major so each newly
            # landed chunk is consumed immediately
            ek = {}
            wk = {}
            for q, ks in enumerate(ENC_RINGS):
                for idx, k in enumerate(ks):
                    ek[k] = enc[q][:, idx, :]
            for p, ks in enumerate(WT_RINGS):
                for idx, k in enumerate(ks):
                    wk[k] = wt[p][:, idx, :]
            # k-major; at k7 finish g1 BEFORE g0 so the DVE cast pipeline
            # (g1 then g0) starts one matmul earlier
            for k in range(KC):
                for g in ((1, 0) if k == KC - 1 else (0, 1)):
                    nc.tensor.matmul(
                        pg[g][:, :],
                        lhsT=wk[k][:, g * 128:(g + 1) * 128],
                        rhs=ek[k],
                        start=(k == 0), stop=(k == KC - 1))

            # tail: cast fp32 PSUM -> fp16 SBUF on DVE (gpsimd cannot read
            # PSUM), then the two output DMAs trigger on the two HWDGE
            # engines in parallel as their cast completes
            nc.vector.tensor_copy(oe[1], pg[1])
            nc.vector.tensor_copy(oe[0], pg[0])
            nc.scalar.dma_start(out_d[1], oe[1])
            nc.sync.dma_start(out_d[0], oe[0])

    nc.compile()
    return nc


def _get_nc():
    nc = _NC_CACHE.get("nc")
    if nc is None:
        nc = _build_nc()
        _NC_CACHE["nc"] = nc
    return nc


def _f16(a):
    return np.ascontiguousarray(a, dtype=np.float16)


def _chunkT(mat):
    # [H(k), F] -> [128, KC, F]: partition p holds k-chunk rows k*128+p
    F = mat.shape[1]
    return np.ascontiguousarray(mat.reshape(KC, 128, F).transpose(1, 0, 2))


def _numpy_fallback(enc, h, keys, Uw, Vw, Ww, prelu_a):
    gate = 1.0 / (1.0 + np.exp(-(enc @ (h + keys).T)))
    pre = (h @ Uw.T + keys @ Vw.T)[:, None, :] + (enc @ Ww.T)[None, :, :]
    cand = np.where(pre >= 0, pre, prelu_a * pre)
    new = h[:, None, :] + gate.T[:, None, :] * cand
    new = np.where(new == 0, np.float32(0.1), new)
    new = np.sign(new).astype(np.float32)
    return new.reshape(NB * B, H)


def kernel(features, states, Uw, Vw, Ww, keys, prelu_a):
    from concourse import bass_utils
    import os

    features = np.asarray(features)
    states = np.asarray(states, dtype=np.float32)
    Uw = np.asarray(Uw, dtype=np.float32)
    Vw = np.asarray(Vw, dtype=np.float32)
    Ww = np.asarray(Ww, dtype=np.float32)
    keys = np.asarray(keys, dtype=np.float32)
    prelu_a = np.asarray(prelu_a, dtype=np.float32)

    enc = np.ascontiguousarray(features[:, 0, :], dtype=np.float32)  # [B, H]
    h = states.reshape(NB, H)

    if np.any(prelu_a <= 0):
        # new is not monotone in ew for a <= 0; never hit in practice
        return _numpy_fallback(enc, h, keys, Uw, Vw, Ww, prelu_a)
    nc = _get_nc()

    # exact thresholds (float64) from the small operands
    e64 = enc.astype(np.float64)
    h64 = h.astype(np.float64)
    k64 = keys.astype(np.float64)
    z = e64 @ (h64 + k64).T                                   # [j, i]
    huv = Uw.astype(np.float64) @ h64.T + Vw.astype(np.float64) @ k64.T
    s = np.where(h64.T > 0, prelu_a.astype(np.float64)[:, None], 1.0)
    with np.errstate(over='ignore'):
        hos = h64.T / s
        nthr = huv + hos * (1.0 + np.exp(-z))
    nthr = np.clip(nthr, -1e30, 1e30).astype(np.float32)      # [H(j), NB]

    # enc.T fp16, chunked [128, KC, B]; each b-half feeds 4 cores
    e3 = _chunkT(_f16(enc.T))
    enc_halves = []
    for bh in range(2):
        eh = e3[:, :, bh * HB:(bh + 1) * HB]
        enc_halves.append({
            f"e{q}": np.ascontiguousarray(eh[:, list(ks), :])
            for q, ks in enumerate(ENC_RINGS)})

    in_maps = []
    for c in range(NCORES):
        jq, bh = c % NJ, c // NJ
        js = slice(jq * JS, (jq + 1) * JS)
        wtc = _chunkT(_f16(Ww[js].T))                         # [128, KC, JS]
        im = {**enc_halves[bh]}
        for p, ks in enumerate(WT_RINGS):
            im[f"w{p}"] = np.ascontiguousarray(wtc[:, list(ks), :])
        in_maps.append(im)

    trace = bool(int(os.environ.get("KERNEL_TRACE", "0")))
    res = bass_utils.run_bass_kernel_spmd(
        nc, in_maps, core_ids=list(range(NCORES)), trace=trace)
    kernel.last_result = res

    # assemble ew [H(j), B] from the per-core fp16 shards
    ew = np.empty((H, B), dtype=np.float32)
    for c in range(NCORES):
        jq, bh = c % NJ, c // NJ
        bs = slice(bh * HB, (bh + 1) * HB)
        j0 = jq * JS
        ew[j0:j0 + 128, bs] = res.results[c]["og0"]
        ew[j0 + 128:j0 + 256, bs] = res.results[c]["og1"]

    # host-side thresholds: out[i, b, j] = +1 iff ew[j, b] >= -nthr[j, i]
    thr = -nthr                                               # [H(j), NB]
    one = np.float32(1.0)
    neg = np.float32(-1.0)
    full = np.empty((NB, B, H), dtype=np.float32)
    for i in range(NB):
        full[i] = np.where(ew >= thr[:, i:i + 1], one, neg).T
    return full.reshape(NB * B, H)


# revision 13
# speedup vs baseline: 1.0924x; 1.0924x over previous
"""Trainium2 Bass kernel for nn_MemoryCell (scatter_memory), v6.

Full-input contract: kernel(**inputs) takes the complete (unsharded) numpy
inputs and returns the full [NB*B, H] output.

Math (B == H == 1024, NB == 5, T == 128):
    enc  = features[:, 0, :]                         # [B, H] - only slice used
    h    = states.reshape(NB, H)
    gate = sigmoid(enc @ (h + keys).T)               # [B, NB]
    pre  = (h @ Uw.T + keys @ Vw.T)[:, None, :] + (enc @ Ww.T)[None, :, :]
    cand = where(pre >= 0, pre, prelu_a * pre)
    new[i, b, j] = h[i, j] + gate[j, i] * cand[i, b, j]   # B==H broadcast quirk
    out  = sign(new) with exact zeros -> +1, reshaped [NB*B, H]

Because gate > 0 and (for prelu slope a > 0) new is monotone in ew =
enc @ Ww.T, each output element is a pure threshold test:

    sign(new[i, b, j]) = +1  iff  ew[j, b] + nthr[j, i] >= 0
    nthr = huv + (h / s) * (1 + exp(-z)),  s = a if h > 0 else 1

v6 split vs the v5 kernel: the device now ONLY computes ew (the one big
matmul) and ships it back as fp16; the host applies the 5 thresholds.
That removes the ACT-table load, the Sign/is_ge tail ops, and the
threshold input DMAs from the device program, and cuts output bytes
2.5x (2 B/elem instead of 5 sign bytes).  A smaller program also means
fewer tile/walrus semaphores, which directly shortens the fixed
end-of-NEFF semaphore-sweep epilogue (~115 ns per semaphore, measured).

Sharding 2D: 4 j-shards (256 features) x 2 b-halves (512 batch).  Per
core: enc half 1.05 MB + Ww j-shard 0.52 MB in, 0.26 MB ew out.

Device program per core:
  * enc ships in 4 k-split rings (k0 | k1 | k2-3 | k4-7) and Ww in 2
    (k0-1 | k2-7) so the first matmul can start after ~256 KB lands
    while the bulk streams.  SP and Scalar drive the two HWDGE gen
    paths; the k1 ring rides gpsimd's SWDGE as a third path.
  * PE: k-major accumulation into two PSUM banks (one per 128-j group),
    16 matmuls of 512 cols.  A few warm-up transposes on a zeroed tile
    keep the PE clock ramping while the first ring lands.
  * Tail: two DVE tensor_copy casts fp32 PSUM -> fp16 SBUF, then two
    output DMAs (SP + Scalar).
"""

import numpy as np

H = 1024
NB = 5
B = 1024
NCORES = 8
NJ = 4                    # j shards
JS = H // NJ              # 256 features per core (2 PE groups of 128)
HB = B // 2               # 512 batch columns per core
KC = H // 128             # 8 contraction chunks
WARMUP = 14

_NC_CACHE = {}

# enc k-chunk split across rings: ring -> list of k chunks
ENC_RINGS = ((0,), (1,), (2, 3), (4, 5), (6, 7))
WT_RINGS = ((0, 1), (2, 3), (4, 5), (6, 7))


def _make_tc_class():
    import concourse.tile as tile
    from concourse.vector_clock import ScopedClock

    class _LeanTileContext(tile.TileContext):
        """TileContext with a minimal teardown: keep the sync-engine drain
        that waits for every DMA/compute lane to reach its final count (so
        the program cannot retire before the output DMAs complete), but
        skip the two all-engine barriers and the semaphore clears.  Engines
        then retire as soon as their own work ends, so the runtime's fixed
        per-engine semaphore-restore epilogue runs concurrently with the
        kernel tail instead of strictly after it."""

        def _drain_and_barrier(self, tick_clock, wait_clock):
            drain_inst = self.nc.sync.drain()
            wait_clock.add_sem_waits(
                drain_inst.ins, ScopedClock({None: tick_clock.global_clock})
            )
            popped = self.nc._tile_sem_poison_stack.pop()
            assert popped is self._sem_poison

    return _LeanTileContext


def _build_nc():
    from concourse import bacc, mybir
    import concourse.tile as tile

    f32 = mybir.dt.float32
    f16 = mybir.dt.float16

    nc = bacc.Bacc("TRN2", debug=False, num_devices=NCORES)

    enc_d = [nc.dram_tensor(f"e{q}", [128, len(ks), HB], f16,
                            kind="ExternalInput").ap()
             for q, ks in enumerate(ENC_RINGS)]
    wt_d = [nc.dram_tensor(f"w{p}", [128, len(ks), JS], f16,
                           kind="ExternalInput").ap()
            for p, ks in enumerate(WT_RINGS)]
    out_d = [nc.dram_tensor(f"og{g}", [128, HB], f16,
                            kind="ExternalOutput").ap() for g in range(2)]

    with _make_tc_class()(nc) as tc:
        with (
            tc.tile_pool(name="res", bufs=1) as res,
            tc.tile_pool(name="ps", bufs=1, space="PSUM") as ps,
        ):
            warm = res.tile([128, 128], f32, name="warm")
            nc.gpsimd.memset(warm, 0.0)

            enc = [res.tile([128, len(ks), HB], f16, name=f"e{q}")
                   for q, ks in enumerate(ENC_RINGS)]
            wt = [res.tile([128, len(ks), JS], f16, name=f"w{p}")
                  for p, ks in enumerate(WT_RINGS)]
            oe = [res.tile([128, HB], f16, name=f"og{g}") for g in range(2)]

            # two balanced HWDGE chains (768 KB each), k-ordered so chunks
            # arrive steadily; each chain interleaves enc and wt so the
            # (enc_k, wt_k) pair for every k lands about together.  SWDGE
            # (gpsimd) completes ~3us late, so it carries NO data ring.
            nc.scalar.dma_start(enc[0], enc_d[0])
            nc.sync.dma_start(wt[0], wt_d[0])
            nc.scalar.dma_start(enc[1], enc_d[1])
            nc.sync.dma_start(enc[2], enc_d[2])
            nc.scalar.dma_start(wt[1], wt_d[1])
            nc.scalar.dma_start(enc[3], enc_d[3])
            nc.sync.dma_start(wt[2], wt_d[2])
            nc.sync.dma_start(enc[4], enc_d[4])
            nc.scalar.dma_start(wt[3], wt_d[3])

            pw = ps.tile([128, 512], f32, name="pw")
            pg = [ps.tile([128, 512], f32, name=f"pg{g}") for g in range(2)]

            # PE warm-up on the zeroed tile: keeps the clock ramping while
            # the first enc/wt rings stream
            for _ in range(WARMUP):
                nc.tensor.transpose(pw[:, 0:128], warm, warm)

            # ew[j, b] = sum_k Ww[j, k] enc[b, k], k-major so each newly
            # landed chunk is consumed immediately
            ek = {}
            wk = {}
            for q, ks in enumerate(ENC_RINGS):
                for idx, k in enumerate(ks):
                    ek[k] = enc[q][:, idx, :]
            for p, ks in enumerate(WT_RINGS):
                for idx, k in enumerate(ks):
                    wk[k] = wt[p][:, idx, :]
            # k-major; at k7 finish g1 BEFORE g0 so the DVE cast pipeline
            # (g1 then g0) starts one matmul earlier.  A dummy transpose
            # after the odd chunks keeps the HAM activity window fed while
            # the next ring's semaphore fires (idle gaps reset the PE
            # clock ramp: 512-col matmuls run 427ns instead of 216ns).
            for k in range(KC):
                for g in ((1, 0) if k == KC - 1 else (0, 1)):
                    nc.tensor.matmul(
                        pg[g][:, :],
                        lhsT=wk[k][:, g * 128:(g + 1) * 128],
                        rhs=ek[k],
                        start=(k == 0), stop=(k == KC - 1))
                if k in (1, 3, 5):
                    nc.tensor.transpose(pw[:, 0:128], warm, warm)

            # tail: cast fp32 PSUM -> fp16 SBUF on DVE (gpsimd cannot read
            # PSUM), then the two output DMAs trigger on the two HWDGE
            # engines in parallel as their cast completes
            nc.vector.tensor_copy(oe[1], pg[1])
            nc.vector.tensor_copy(oe[0], pg[0])
            nc.scalar.dma_start(out_d[1], oe[1])
            nc.sync.dma_start(out_d[0], oe[0])

    nc.compile()
    return nc


def _get_nc():
    nc = _NC_CACHE.get("nc")
    if nc is None:
        nc = _build_nc()
        _NC_CACHE["nc"] = nc
    return nc


def _f16(a):
    return np.ascontiguousarray(a, dtype=np.float16)


def _chunkT(mat):
    # [H(k), F] -> [128, KC, F]: partition p holds k-chunk rows k*128+p
    F = mat.shape[1]
    return np.ascontiguousarray(mat.reshape(KC, 128, F).transpose(1, 0, 2))


def _numpy_fallback(enc, h, keys, Uw, Vw, Ww, prelu_a):
    gate = 1.0 / (1.0 + np.exp(-(enc @ (h + keys).T)))
    pre = (h @ Uw.T + keys @ Vw.T)[:, None, :] + (enc @ Ww.T)[None, :, :]
    cand = np.where(pre >= 0, pre, prelu_a * pre)
    new = h[:, None, :] + gate.T[:, None, :] * cand
    new = np.where(new == 0, np.float32(0.1), new)
    new = np.sign(new).astype(np.float32)
    return new.reshape(NB * B, H)


def kernel(features, states, Uw, Vw, Ww, keys, prelu_a):
    from concourse import bass_utils
    import os

    features = np.asarray(features)
    states = np.asarray(states, dtype=np.float32)
    Uw = np.asarray(Uw, dtype=np.float32)
    Vw = np.asarray(Vw, dtype=np.float32)
    Ww = np.asarray(Ww, dtype=np.float32)
    keys = np.asarray(keys, dtype=np.float32)
    prelu_a = np.asarray(prelu_a, dtype=np.float32)

    enc = np.ascontiguousarray(features[:, 0, :], dtype=np.float32)  # [B, H]
    h = states.reshape(NB, H)

    if np.any(prelu_a <= 0):
        # new is not monotone in ew for a <= 0; never hit in practice
        return _numpy_fallback(enc, h, keys, Uw, Vw, Ww, prelu_a)
    nc = _get_nc()

    # exact thresholds (float64) from the small operands
    e64 = enc.astype(np.float64)
    h64 = h.astype(np.float64)
    k64 = keys.astype(np.float64)
    z = e64 @ (h64 + k64).T                                   # [j, i]
    huv = Uw.astype(np.float64) @ h64.T + Vw.astype(np.float64) @ k64.T
    s = np.where(h64.T > 0, prelu_a.astype(np.float64)[:, None], 1.0)
    with np.errstate(over='ignore'):
        hos = h64.T / s
        nthr = huv + hos * (1.0 + np.exp(-z))
    nthr = np.clip(nthr, -1e30, 1e30).astype(np.float32)      # [H(j), NB]

    # enc.T fp16, chunked [128, KC, B]; each b-half feeds 4 cores
    e3 = _chunkT(_f16(enc.T))
    enc_halves = []
    for bh in range(2):
        eh = e3[:, :, bh * HB:(bh + 1) * HB]
        enc_halves.append({
            f"e{q}": np.ascontiguousarray(eh[:, list(ks), :])
            for q, ks in enumerate(ENC_RINGS)})

    in_maps = []
    for c in range(NCORES):
        jq, bh = c % NJ, c // NJ
        js = slice(jq * JS, (jq + 1) * JS)
        wtc = _chunkT(_f16(Ww[js].T))                         # [128, KC, JS]
        im = {**enc_halves[bh]}
        for p, ks in enumerate(WT_RINGS):
            im[f"w{p}"] = np.ascontiguousarray(wtc[:, list(ks), :])
        in_maps.append(im)

    trace = bool(int(os.environ.get("KERNEL_TRACE", "0")))
    res = bass_utils.run_bass_kernel_spmd(
        nc, in_maps, core_ids=list(range(NCORES)), trace=trace)
    kernel.last_result = res

    # assemble ew [H(j), B] from the per-core fp16 shards
    ew = np.empty((H, B), dtype=np.float32)
    for c in range(NCORES):
        jq, bh = c % NJ, c // NJ
        bs = slice(bh * HB, (bh + 1) * HB)
        j0 = jq * JS
        ew[j0:j0 + 128, bs] = res.results[c]["og0"]
        ew[j0 + 128:j0 + 256, bs] = res.results[c]["og1"]

    # host-side thresholds: out[i, b, j] = +1 iff ew[j, b] >= -nthr[j, i]
    thr = -nthr                                               # [H(j), NB]
    one = np.float32(1.0)
    neg = np.float32(-1.0)
    full = np.empty((NB, B, H), dtype=np.float32)
    for i in range(NB):
        full[i] = np.where(ew >= thr[:, i:i + 1], one, neg).T
    return full.reshape(NB * B, H)


# revision 17
# speedup vs baseline: 1.2114x; 1.1089x over previous
"""Trainium2 Bass kernel for nn_MemoryCell (scatter_memory), v6.

Full-input contract: kernel(**inputs) takes the complete (unsharded) numpy
inputs and returns the full [NB*B, H] output.

Math (B == H == 1024, NB == 5, T == 128):
    enc  = features[:, 0, :]                         # [B, H] - only slice used
    h    = states.reshape(NB, H)
    gate = sigmoid(enc @ (h + keys).T)               # [B, NB]
    pre  = (h @ Uw.T + keys @ Vw.T)[:, None, :] + (enc @ Ww.T)[None, :, :]
    cand = where(pre >= 0, pre, prelu_a * pre)
    new[i, b, j] = h[i, j] + gate[j, i] * cand[i, b, j]   # B==H broadcast quirk
    out  = sign(new) with exact zeros -> +1, reshaped [NB*B, H]

Because gate > 0 and (for prelu slope a > 0) new is monotone in ew =
enc @ Ww.T, each output element is a pure threshold test:

    sign(new[i, b, j]) = +1  iff  ew[j, b] + nthr[j, i] >= 0
    nthr = huv + (h / s) * (1 + exp(-z)),  s = a if h > 0 else 1

v6 split vs the v5 kernel: the device now ONLY computes ew (the one big
matmul) and ships it back as fp16; the host applies the 5 thresholds.
That removes the ACT-table load, the Sign/is_ge tail ops, and the
threshold input DMAs from the device program, and cuts output bytes
2.5x (2 B/elem instead of 5 sign bytes).  A smaller program also means
fewer tile/walrus semaphores, which directly shortens the fixed
end-of-NEFF semaphore-sweep epilogue (~115 ns per semaphore, measured).

Sharding 2D: 4 j-shards (256 features) x 2 b-halves (512 batch).  Per
core: enc half 1.05 MB + Ww j-shard 0.52 MB in, 0.26 MB ew out.

Device program per core:
  * enc ships in 4 k-split rings (k0 | k1 | k2-3 | k4-7) and Ww in 2
    (k0-1 | k2-7) so the first matmul can start after ~256 KB lands
    while the bulk streams.  SP and Scalar drive the two HWDGE gen
    paths; the k1 ring rides gpsimd's SWDGE as a third path.
  * PE: k-major accumulation into two PSUM banks (one per 128-j group),
    16 matmuls of 512 cols.  A few warm-up transposes on a zeroed tile
    keep the PE clock ramping while the first ring lands.
  * Tail: two DVE tensor_copy casts fp32 PSUM -> fp16 SBUF, then two
    output DMAs (SP + Scalar).
"""

import numpy as np

H = 1024
NB = 5
B = 1024
NCORES = 8
NJ = 4                    # j shards
JS = H // NJ              # 256 features per core (2 PE groups of 128)
HB = B // 2               # 512 batch columns per core
KC = H // 128             # 8 contraction chunks
WARMUP = 18

_NC_CACHE = {}

TW = HB + JS              # 768: packed ring = enc k-chunk (512) + wt (256)


def _make_tc_class():
    import concourse.tile as tile
    from concourse.vector_clock import ScopedClock

    class _LeanTileContext(tile.TileContext):
        """TileContext with a minimal teardown: keep the sync-engine drain
        that waits for every DMA/compute lane to reach its final count (so
        the program cannot retire before the output DMAs complete), but
        skip the two all-engine barriers and the semaphore clears.  Engines
        then retire as soon as their own work ends, so the runtime's fixed
        per-engine semaphore-restore epilogue runs concurrently with the
        kernel tail instead of strictly after it."""

        def _drain_and_barrier(self, tick_clock, wait_clock):
            drain_inst = self.nc.sync.drain()
            wait_clock.add_sem_waits(
                drain_inst.ins, ScopedClock({None: tick_clock.global_clock})
            )
            popped = self.nc._tile_sem_poison_stack.pop()
            assert popped is self._sem_poison

    return _LeanTileContext


def _build_nc():
    from concourse import bacc, mybir
    import concourse.tile as tile

    f32 = mybir.dt.float32
    f16 = mybir.dt.float16

    nc = bacc.Bacc("TRN2", debug=False, num_devices=NCORES)

    t_d = [nc.dram_tensor(f"t{k}", [128, TW], f16,
                          kind="ExternalInput").ap() for k in range(KC)]
    out_d = [nc.dram_tensor(f"og{g}", [128, HB], f16,
                            kind="ExternalOutput").ap() for g in range(2)]

    with _make_tc_class()(nc) as tc:
        with (
            tc.tile_pool(name="res", bufs=1) as res,
            tc.tile_pool(name="ps", bufs=1, space="PSUM") as ps,
        ):
            warm = res.tile([128, 128], f32, name="warm")
            nc.gpsimd.memset(warm, 0.0)

            tk = [res.tile([128, TW], f16, name=f"t{k}") for k in range(KC)]
            oe = [res.tile([128, HB], f16, name=f"og{g}") for g in range(2)]

            # one packed (enc_k | wt_k) ring per contraction chunk: a
            # single completion semaphore gates both operands of chunk k.
            # Even chunks ride the Scalar HWDGE chain, odd chunks ride
            # Sync, so the two FIFOs stream in parallel and chunks land
            # in consumption order.  SWDGE (gpsimd) completes ~3us late,
            # so it carries NO data ring.
            for k in range(KC):
                eng = nc.scalar if k % 2 == 0 else nc.sync
                eng.dma_start(tk[k], t_d[k])

            pw = ps.tile([128, 512], f32, name="pw")
            pg = [ps.tile([128, 512], f32, name=f"pg{g}") for g in range(2)]

            # PE warm-up on the zeroed tile: keeps the clock ramping while
            # the first enc/wt rings stream
            for _ in range(WARMUP):
                nc.tensor.transpose(pw[:, 0:128], warm, warm)

            # ew[j, b] = sum_k Ww[j, k] enc[b, k], k-major so each newly
            # landed chunk is consumed immediately
            ek = {k: tk[k][:, 0:HB] for k in range(KC)}
            wk = {k: tk[k][:, HB:TW] for k in range(KC)}
            # k-major; at k7 finish g1 BEFORE g0 so the DVE cast pipeline
            # (g1 then g0) starts one matmul earlier.  A dummy transpose
            # after the odd chunks keeps the HAM activity window fed while
            # the next ring's semaphore fires (idle gaps reset the PE
            # clock ramp: 512-col matmuls run 427ns instead of 216ns).
            for k in range(KC):
                for g in ((1, 0) if k == KC - 1 else (0, 1)):
                    nc.tensor.matmul(
                        pg[g][:, :],
                        lhsT=wk[k][:, g * 128:(g + 1) * 128],
                        rhs=ek[k],
                        start=(k == 0), stop=(k == KC - 1))
                if k in (1, 3, 5):
                    nc.tensor.transpose(pw[:, 0:128], warm, warm)

            # tail: cast fp32 PSUM -> fp16 SBUF on DVE (gpsimd cannot read
            # PSUM), then the two output DMAs trigger on the two HWDGE
            # engines in parallel as their cast completes
            nc.vector.tensor_copy(oe[1], pg[1])
            nc.vector.tensor_copy(oe[0], pg[0])
            nc.scalar.dma_start(out_d[1], oe[1])
            nc.sync.dma_start(out_d[0], oe[0])

    nc.compile()
    return nc


def _get_nc():
    nc = _NC_CACHE.get("nc")
    if nc is None:
        nc = _build_nc()
        _NC_CACHE["nc"] = nc
    return nc


def _f16(a):
    return np.ascontiguousarray(a, dtype=np.float16)


def _chunkT(mat):
    # [H(k), F] -> [128, KC, F]: partition p holds k-chunk rows k*128+p
    F = mat.shape[1]
    return np.ascontiguousarray(mat.reshape(KC, 128, F).transpose(1, 0, 2))


def _numpy_fallback(enc, h, keys, Uw, Vw, Ww, prelu_a):
    gate = 1.0 / (1.0 + np.exp(-(enc @ (h + keys).T)))
    pre = (h @ Uw.T + keys @ Vw.T)[:, None, :] + (enc @ Ww.T)[None, :, :]
    cand = np.where(pre >= 0, pre, prelu_a * pre)
    new = h[:, None, :] + gate.T[:, None, :] * cand
    new = np.where(new == 0, np.float32(0.1), new)
    new = np.sign(new).astype(np.float32)
    return new.reshape(NB * B, H)


def kernel(features, states, Uw, Vw, Ww, keys, prelu_a):
    from concourse import bass_utils
    import os

    features = np.asarray(features)
    states = np.asarray(states, dtype=np.float32)
    Uw = np.asarray(Uw, dtype=np.float32)
    Vw = np.asarray(Vw, dtype=np.float32)
    Ww = np.asarray(Ww, dtype=np.float32)
    keys = np.asarray(keys, dtype=np.float32)
    prelu_a = np.asarray(prelu_a, dtype=np.float32)

    enc = np.ascontiguousarray(features[:, 0, :], dtype=np.float32)  # [B, H]
    h = states.reshape(NB, H)

    if np.any(prelu_a <= 0):
        # new is not monotone in ew for a <= 0; never hit in practice
        return _numpy_fallback(enc, h, keys, Uw, Vw, Ww, prelu_a)
    nc = _get_nc()

    # exact thresholds (float64) from the small operands
    e64 = enc.astype(np.float64)
    h64 = h.astype(np.float64)
    k64 = keys.astype(np.float64)
    z = e64 @ (h64 + k64).T                                   # [j, i]
    huv = Uw.astype(np.float64) @ h64.T + Vw.astype(np.float64) @ k64.T
    s = np.where(h64.T > 0, prelu_a.astype(np.float64)[:, None], 1.0)
    with np.errstate(over='ignore'):
        hos = h64.T / s
        nthr = huv + hos * (1.0 + np.exp(-z))
    nthr = np.clip(nthr, -1e30, 1e30).astype(np.float32)      # [H(j), NB]

    # enc.T fp16, chunked [128, KC, B]; each b-half feeds 4 cores
    e3 = _chunkT(_f16(enc.T))
    wtcs = [_chunkT(_f16(Ww[jq * JS:(jq + 1) * JS].T))        # [128, KC, JS]
            for jq in range(NJ)]

    in_maps = []
    for c in range(NCORES):
        jq, bh = c % NJ, c // NJ
        eh = e3[:, :, bh * HB:(bh + 1) * HB]
        im = {f"t{k}": np.ascontiguousarray(
                  np.concatenate([eh[:, k, :], wtcs[jq][:, k, :]], axis=1))
              for k in range(KC)}
        in_maps.append(im)

    trace = bool(int(os.environ.get("KERNEL_TRACE", "0")))
    res = bass_utils.run_bass_kernel_spmd(
        nc, in_maps, core_ids=list(range(NCORES)), trace=trace)
    kernel.last_result = res

    # assemble ew [H(j), B] from the per-core fp16 shards
    ew = np.empty((H, B), dtype=np.float32)
    for c in range(NCORES):
        jq, bh = c % NJ, c // NJ
        bs = slice(bh * HB, (bh + 1) * HB)
        j0 = jq * JS
        ew[j0:j0 + 128, bs] = res.results[c]["og0"]
        ew[j0 + 128:j0 + 256, bs] = res.results[c]["og1"]

    # host-side thresholds: out[i, b, j] = +1 iff ew[j, b] >= -nthr[j, i]
    thr = -nthr                                               # [H(j), NB]
    one = np.float32(1.0)
    neg = np.float32(-1.0)
    full = np.empty((NB, B, H), dtype=np.float32)
    for i in range(NB):
        full[i] = np.where(ew >= thr[:, i:i + 1], one, neg).T
    return full.reshape(NB * B, H)
